# revision 2
# baseline (speedup 1.0000x reference)
"""EmmaSAGELayer GNN message-passing kernel for 8 Trainium2 NeuronCores.

Strategy (graph/data parallel per sharding hint):
- Nodes are sharded across 8 cores by destination: core k owns dst rows
  [k*12500, (k+1)*12500). Edges are routed to the core owning their dst.
- x (src features) is replicated to every core's HBM as bf16 hi/lo pairs
  (hi = bf16(x), lo = bf16(x - hi); 512B/row, f32-accurate to ~1e-5 after
  the PSUM f32 accumulation).
- Each core: per-edge dma_gather of x[src] rows (int16 indices force 4
  src-range buckets of 25600 rows), segment-sum via one-hot matmuls
  accumulated in PSUM (lhsT = one-hot of dst slot built on DVE from
  per-edge offsets, rhs = gathered bf16 rows), normalize by 1/deg
  (deg computed on-device from row-pointer diffs), PE transpose, then the
  fused linear out = h @ W1^T + x @ W2^T + b computed column-major, with
  the output written transposed and unshuffled on the host.
- Destinations are bin-packed into 100 groups of <=128 dsts and <=2048
  edges (balances the per-group chunk schedule so one compiled program
  serves all 8 cores; per-core variation is absorbed by padding slots
  whose one-hot offset 255 matches no lane).
"""
import heapq

import numpy as np
import ml_dtypes

N_NODES = 100000
N_EDGES = 1600000
C = 128
NCORES = 8
ND = N_NODES // NCORES          # 12500 dst rows per core
NG = 100                        # dst groups per core
PC = 128                        # dst slots per group
NDPAD = NG * PC                 # 12800
NBUCKET = 4
BUCKET = 25600                  # src rows per gather bucket (int16 range)
GROUP_EDGE_CAP = 2048           # bin-pack edge capacity per group


def _wrap_idx(v):
    """dma_gather index layout: [16, n/16] wrap replicated to 128 parts."""
    w = v.reshape(-1, 16).T                    # [16, n/16]
    return np.tile(w, (8, 1))                  # [128, n/16]


def _pack_core(edge_dst_l, edge_src, kb):
    """Bin-pack one core's dsts into NG groups; build index/offset arrays.

    Returns (idx16 [128, NG*4, KB*8], offs [128, NG*4*KB], rp0, rp1,
             dstof [NDPAD]) with kb chunks per (group, bucket).
    """
    deg = np.bincount(edge_dst_l, minlength=ND).astype(np.int64)
    order = np.argsort(-deg, kind="stable")
    # least-loaded-first bin pack, <=PC dsts and <=GROUP_EDGE_CAP edges/bin
    heap = [(0, g) for g in range(NG)]
    heapq.heapify(heap)
    ndst = np.zeros(NG, np.int64)
    group_of = np.empty(ND, np.int64)
    slot_of = np.empty(ND, np.int64)
    spill = []
    for d in order:
        placed = False
        tmp = []
        while heap:
            load, g = heapq.heappop(heap)
            if ndst[g] < PC and load + deg[d] <= GROUP_EDGE_CAP:
                group_of[d] = g
                slot_of[d] = ndst[g]
                ndst[g] += 1
                heapq.heappush(heap, (load + deg[d], g))
                placed = True
                break
            tmp.append((load, g))
        for item in tmp:
            heapq.heappush(heap, item)
        if not placed:
            spill.append(d)
    assert not spill, f"bin packing failed for {len(spill)} dsts"

    g_e = group_of[edge_dst_l]
    b_e = edge_src // BUCKET
    s_e = slot_of[edge_dst_l]
    seg = g_e * NBUCKET + b_e                       # segment id per edge
    order_e = np.lexsort((edge_src, seg))
    seg_s = seg[order_e]
    src_s = edge_src[order_e]
    slot_s = s_e[order_e]
    # rank within segment
    counts = np.bincount(seg_s, minlength=NG * NBUCKET)
    assert counts.max() <= kb * 128, f"segment overflow {counts.max()}"
    seg_start = np.zeros(NG * NBUCKET, np.int64)
    seg_start[1:] = np.cumsum(counts)[:-1]
    rank = np.arange(len(seg_s)) - seg_start[seg_s]

    cap = kb * 128
    idx16 = np.zeros((NG * NBUCKET, cap), np.int16)          # pad idx 0
    offs = np.full((NG * NBUCKET, cap), 255.0, np.float32)   # pad slot 255
    pos = seg_s * cap + rank
    idx16.reshape(-1)[pos] = (src_s - (seg_s % NBUCKET) * BUCKET).astype(np.int16)
    offs.reshape(-1)[pos] = slot_s.astype(np.float32)

    # wrapped idx: [NG*4, 128, cap//16] -> transpose to [128, NG*4, cap//16]
    idx_w = np.stack([_wrap_idx(idx16[i]) for i in range(NG * NBUCKET)])
    idx_w = np.ascontiguousarray(idx_w.transpose(1, 0, 2))
    # offs emission order: (g, b, k) chunk -> [128, NG*4*KB]
    offs_t = np.ascontiguousarray(
        offs.reshape(NG * NBUCKET * kb, 128).T).astype(np.float32)

    # per (slot, group) degree as row-pointer pair (device computes diff)
    deg_sg = np.zeros((PC, NG), np.int32)
    deg_sg[slot_of, group_of] = deg.astype(np.int32)
    csum = np.cumsum(deg_sg.reshape(-1)).astype(np.int32).reshape(PC, NG)
    rp1 = csum
    rp0 = (csum - deg_sg).astype(np.int32)

    dstof = np.full(NDPAD, -1, np.int64)
    dstof[group_of * PC + slot_of] = np.arange(ND)
    return idx_w, offs_t, rp0, rp1, dstof


def _build_program(kb):
    from contextlib import ExitStack
    import concourse.bass as bass
    import concourse.bacc as bacc
    import concourse.mybir as mybir
    import concourse.tile as tile

    nc = bacc.Bacc("TRN2", target_bir_lowering=False, num_swdge_queues=4)
    f32, bf16, i16, i32 = (mybir.dt.float32, mybir.dt.bfloat16,
                           mybir.dt.int16, mybir.dt.int32)
    cols = kb * 128 // 16
    d_xcat = nc.declare_dram_parameter("xcat", [NBUCKET * BUCKET, C], f32,
                                       isOutput=False)
    d_idx = nc.declare_dram_parameter("idx", [128, NG * NBUCKET, cols], i16,
                                      isOutput=False)
    d_offs = nc.declare_dram_parameter("offs", [128, NG * NBUCKET * kb], f32,
                                       isOutput=False)
    d_rp0 = nc.declare_dram_parameter("rp0", [128, NG], i32, isOutput=False)
    d_rp1 = nc.declare_dram_parameter("rp1", [128, NG], i32, isOutput=False)
    d_xt = nc.declare_dram_parameter("xt", [128, NDPAD], f32, isOutput=False)
    d_iota = nc.declare_dram_parameter("iota", [128, 128], bf16, isOutput=False)
    d_ident = nc.declare_dram_parameter("ident", [128, 128], f32, isOutput=False)
    d_w1t = nc.declare_dram_parameter("w1t", [128, 128], f32, isOutput=False)
    d_w2t = nc.declare_dram_parameter("w2t", [128, 128], f32, isOutput=False)
    d_bias = nc.declare_dram_parameter("bias", [128, 1], f32, isOutput=False)
    d_out = nc.declare_dram_parameter("outT", [128, NDPAD], f32, isOutput=True)

    with tile.TileContext(nc) as tc, ExitStack() as ctx:
        consts = ctx.enter_context(tc.tile_pool(name="consts", bufs=1))
        gpool = ctx.enter_context(tc.tile_pool(name="gather", bufs=8))
        spool = ctx.enter_context(tc.tile_pool(name="onehot", bufs=4))
        hpool = ctx.enter_context(tc.tile_pool(name="h", bufs=3))
        opool = ctx.enter_context(tc.tile_pool(name="outbuf", bufs=2))
        ps_agg = ctx.enter_context(tc.tile_pool(name="psagg", bufs=3, space="PSUM"))
        ps_t = ctx.enter_context(tc.tile_pool(name="pst", bufs=2, space="PSUM"))
        ps_o = ctx.enter_context(tc.tile_pool(name="pso", bufs=2, space="PSUM"))

        t_iota = consts.tile([128, 128], bf16)
        nc.sync.dma_start(t_iota[:], d_iota[:])
        t_ident = consts.tile([128, 128], f32)
        nc.sync.dma_start(t_ident[:], d_ident[:])
        t_w1t = consts.tile([128, 128], f32)
        nc.sync.dma_start(t_w1t[:], d_w1t[:])
        t_w2t = consts.tile([128, 128], f32)
        nc.sync.dma_start(t_w2t[:], d_w2t[:])
        t_bias = consts.tile([128, 1], f32)
        nc.sync.dma_start(t_bias[:], d_bias[:])
        t_idx = consts.tile([128, NG * NBUCKET, cols], i16)
        nc.sync.dma_start(t_idx[:], d_idx[:])
        t_offs = consts.tile([128, NG * NBUCKET * kb], f32)
        nc.sync.dma_start(t_offs[:], d_offs[:])
        t_xt = consts.tile([128, NDPAD], f32)
        nc.sync.dma_start(t_xt[:], d_xt[:])

        # inv = 1 / max(rp1 - rp0, 1)  (deg==0 rows have agg==0 anyway)
        t_rp0 = consts.tile([128, NG], i32)
        nc.sync.dma_start(t_rp0[:], d_rp0[:])
        t_rp1 = consts.tile([128, NG], i32)
        nc.sync.dma_start(t_rp1[:], d_rp1[:])
        t_degi = consts.tile([128, NG], i32)
        nc.vector.tensor_tensor(out=t_degi[:], in0=t_rp1[:], in1=t_rp0[:],
                                op=mybir.AluOpType.subtract)
        t_deg = consts.tile([128, NG], f32)
        nc.vector.tensor_copy(out=t_deg[:], in_=t_degi[:])
        nc.vector.tensor_scalar_max(t_deg[:], t_deg[:], 1.0)
        t_inv = consts.tile([128, NG], f32)
        nc.vector.reciprocal(t_inv[:], t_deg[:])

        nmm = NBUCKET * kb
        for g in range(NG):
            ps = ps_agg.tile([128, 256], f32, space="PSUM")
            mm = 0
            for b in range(NBUCKET):
                gt = gpool.tile([128, kb, C], f32)
                nc.gpsimd.dma_gather(
                    out_ap=gt[:],
                    in_ap=d_xcat[b * BUCKET:(b + 1) * BUCKET, :],
                    idxs_ap=t_idx[:, g * NBUCKET + b, :],
                    num_idxs=kb * 128,
                    num_idxs_reg=kb * 128,
                    elem_size=C,
                    queue_num=b,
                )
                for k in range(kb):
                    t = (g * NBUCKET + b) * kb + k
                    S = spool.tile([128, 128], bf16)
                    nc.vector.tensor_scalar(
                        out=S[:], in0=t_iota[:],
                        scalar1=t_offs[:, t:t + 1], scalar2=None,
                        op0=mybir.AluOpType.is_equal,
                    )
                    nc.tensor.matmul(
                        out=ps[:], lhsT=S[:],
                        rhs=gt[:, k, :].bitcast(bf16),
                        start=(mm == 0), stop=(mm == nmm - 1),
                    )
                    mm += 1
            # h = (hi + lo) * inv   (one PSUM operand per DVE op)
            lo = hpool.tile([128, 128], f32, tag="lo")
            nc.scalar.activation(out=lo[:], in_=ps[:, 128:256],
                                 func=mybir.ActivationFunctionType.Identity)
            h = hpool.tile([128, 128], f32, tag="h")
            nc.vector.tensor_add(out=h[:], in0=ps[:, 0:128], in1=lo[:])
            nc.vector.tensor_scalar_mul(h[:], h[:], t_inv[:, g:g + 1])
            pst = ps_t.tile([128, 128], f32, space="PSUM")
            nc.tensor.transpose(out=pst[:], in_=h[:], identity=t_ident[:])
            hT = hpool.tile([128, 128], f32, tag="ht")
            nc.vector.tensor_copy(out=hT[:], in_=pst[:])
            pso = ps_o.tile([128, 128], f32, space="PSUM")
            nc.tensor.matmul(out=pso[:], lhsT=t_w1t[:], rhs=hT[:],
                             start=True, stop=False)
            nc.tensor.matmul(out=pso[:], lhsT=t_w2t[:],
                             rhs=t_xt[:, g * 128:(g + 1) * 128],
                             start=False, stop=True)
            if g % 8 == 0:
                ob = opool.tile([128, 8 * 128], f32)
            nc.scalar.activation(
                out=ob[:, (g % 8) * 128:(g % 8 + 1) * 128], in_=pso[:],
                func=mybir.ActivationFunctionType.Identity,
                bias=t_bias[:], scale=1.0,
            )
            if g % 8 == 7 or g == NG - 1:
                g0 = g - (g % 8)
                nc.sync.dma_start(
                    d_out[:, g0 * 128:(g + 1) * 128],
                    ob[:, :(g - g0 + 1) * 128])
    nc.compile()
    return nc


def kernel(x, edge_src, edge_dst, weight, bias):
    import axon_profile_shim  # noqa: F401  (registers ntff hook if missing)
    from concourse.bass_utils import run_bass_kernel_spmd

    x = np.asarray(x)
    edge_src = np.asarray(edge_src).astype(np.int64)
    edge_dst = np.asarray(edge_dst).astype(np.int64)
    weight = np.asarray(weight)
    bias_v = np.asarray(bias)

    # hi/lo bf16 split of x, padded to NBUCKET*BUCKET rows
    xh = x.astype(ml_dtypes.bfloat16)
    xl = (x - xh.astype(np.float32)).astype(ml_dtypes.bfloat16)
    xcat = np.zeros((NBUCKET * BUCKET, 2 * C), ml_dtypes.bfloat16)
    xcat[:N_NODES, :C] = xh
    xcat[:N_NODES, C:] = xl
    xcat_f32 = xcat.view(np.float32)

    core_of = edge_dst // ND
    packs = []
    kb_need = 0
    per_core = []
    for k in range(NCORES):
        m = core_of == k
        es, ed = edge_src[m], edge_dst[m] - k * ND
        per_core.append((es, ed))
        deg = np.bincount(ed, minlength=ND)
        kb_need = max(kb_need, 5)
    kb = 5
    # build per-core arrays; bump kb if any segment overflows
    while True:
        try:
            packs = [_pack_core(ed, es, kb) for (es, ed) in per_core]
            break
        except AssertionError:
            kb += 1
            if kb > 8:
                raise

    nc = _build_program(kb)

    iota = np.tile(np.arange(128, dtype=np.float32), (128, 1)).astype(
        ml_dtypes.bfloat16)
    ident = np.eye(128, dtype=np.float32)
    w1t = np.ascontiguousarray(weight[:, :C].T)
    w2t = np.ascontiguousarray(weight[:, C:].T)

    in_maps = []
    for k in range(NCORES):
        idx_w, offs_t, rp0, rp1, dstof = packs[k]
        xt = np.zeros((128, NDPAD), np.float32)
        valid = dstof >= 0
        xt[:, valid] = x[k * ND + dstof[valid]].T
        in_maps.append(dict(
            xcat=xcat_f32, idx=idx_w, offs=offs_t, rp0=rp0, rp1=rp1,
            xt=xt, iota=iota, ident=ident, w1t=w1t, w2t=w2t,
            bias=np.ascontiguousarray(bias_v.reshape(128, 1)),
        ))

    res = run_bass_kernel_spmd(nc, in_maps, core_ids=list(range(NCORES)),
                               trace=True)
    global LAST_EXEC_NS
    LAST_EXEC_NS = res.exec_time_ns

    out = np.empty((N_NODES, C), np.float32)
    for k in range(NCORES):
        outT = np.asarray(res.results[k]["outT"])          # [128, NDPAD]
        dstof = packs[k][4]
        valid = dstof >= 0
        out[k * ND + dstof[valid]] = outT[:, valid].T
    return out
